# revision 4
# baseline (speedup 1.0000x reference)
# Trainium2 Bass kernel: Llama-style attention block (GQA + RoPE + causal),
# tensor-parallel across heads on 8 NeuronCores.
#
# Full-shape contract: kernel(**inputs) takes the unsharded numpy inputs and
# returns the full [B, S, HID] float32 output.
#
# Sharding strategy (per core i of 8):
#   - 4 query heads (rows i*512:(i+1)*512 of Wq) + 1 kv head (rows i*128.. of Wk/Wv)
#   - Wo is sharded row-wise (its columns i*512:(i+1)*512); each core emits a
#     partial [B,S,HID] product (bf16) which the host sums at gather time.
# All weights/activations are pre-transposed and pre-tiled on the host so the
# device kernel needs zero on-chip transposes of activations:
#   matmul(out[M,N], lhsT[K,M], rhs[K,N]) contracts over the partition dim K.
# Compute in bf16 (fp32 PSUM accumulation); scores stay transposed [k, q];
# softmax is unnormalized (scores are O(10), exp safe in fp32); the softmax
# row-sum is accumulated on DVE during the kt loop, then sum+broadcast happen
# in ONE [128,128]-ones matmul, reciprocal via the fast DVE approx, folded in
# at the attention-output eviction. The causal diagonal band computes only
# live columns.
#
# Phase structure per (b, tci):  KV-sweep (2 psums) -> Q-sweep (4 psums),
# so the rope/copy evictions of each sweep hide behind the other sweep's
# matmuls instead of stalling the next chunk's accumulation.

import os
import sys
from contextlib import ExitStack

for _p in ("/opt/trn_rl_repo", "/root/.axon_site/_ro/trn_rl_repo"):
    if os.path.isdir(_p) and _p not in sys.path:
        sys.path.append(_p)

import ml_dtypes
import numpy as np

import concourse.bass as bass
import concourse.mybir as mybir
import concourse.tile as tile
from concourse import bacc
from concourse.bass_utils import run_bass_kernel_spmd

BF16 = mybir.dt.bfloat16
F32 = mybir.dt.float32
NEG = -1.0e9
N_CORES = 8


def build_core_kernel(B, S, HID, QH=4, D=128, QCH=512):
    """SPMD per-core program. QH query heads + 1 kv head per core.

    DRAM parameter layouts (host pre-tiles everything):
      ht   [B, KT, TC, 128, QCH]  bf16  hidden^T tiles: [b,kt,tc,i,j] = hidden[b, tc*QCH+j, kt*128+i]
      trig [B, 2, D, S]           bf16  cos^T / sin^T
      wq   [KT, 128, QH*D]        bf16  Wq_core^T tiles (contraction rows blocked by 128)
      wk   [KT, 128, D]           bf16
      wv   [KT, 128, D]           bf16
      wo   [QH, D, HID]           bf16  Wo_core^T rows blocked per head
      mask [128, 128]             f32   additive causal triangle
      eye  [128, 128]             bf16
      out  [B, TT, HC, 128, QCH]  bf16  partial output tiles
    """
    FS = QH * D          # per-core feature slice of the qkv/attn space
    KT = HID // 128      # contraction tiles for projections
    TC = S // QCH        # 512-token chunks
    TT = S // 128        # 128-token tiles
    HC = HID // QCH      # output hid chunks
    KPQ = QCH // 128     # k-tiles per q-chunk (diagonal band width)
    HALF = D // 2
    SC = float(1.0 / np.sqrt(D))
    EXP = mybir.ActivationFunctionType.Exp

    # Bacc (not plain Bass): its compile pipeline splits multi-sem waits into
    # EventSemaphore instructions — the DMA DIRECT2D struct has one wait slot.
    nc = bacc.Bacc(None)
    ht = nc.declare_dram_parameter("ht", [B, KT, TC, 128, QCH], BF16, isOutput=False)
    trig = nc.declare_dram_parameter("trig", [B, 2, D, S], BF16, isOutput=False)
    wq = nc.declare_dram_parameter("wq", [KT, 128, FS], BF16, isOutput=False)
    wk = nc.declare_dram_parameter("wk", [KT, 128, D], BF16, isOutput=False)
    wv = nc.declare_dram_parameter("wv", [KT, 128, D], BF16, isOutput=False)
    wo = nc.declare_dram_parameter("wo", [QH, D, HID], BF16, isOutput=False)
    mask = nc.declare_dram_parameter("mask", [128, 128], F32, isOutput=False)
    eye = nc.declare_dram_parameter("eye", [128, 128], BF16, isOutput=False)
    out = nc.declare_dram_parameter("out", [B, TT, HC, 128, QCH], BF16, isOutput=True)

    with ExitStack() as ctx:
        tc = ctx.enter_context(tile.TileContext(nc))
        pool = lambda name, bufs, space=None: ctx.enter_context(
            tc.tile_pool(name=name, bufs=bufs, **({"space": space} if space else {}))
        )
        p_w = pool("p_w", 1)          # weights + constants, loaded once
        p_ht = pool("p_ht", 33)       # hidden^T tiles, resident across both sweeps
        p_qt = pool("p_qt", QH)       # per-head Q^T [D, S] bf16
        p_kt = pool("p_kt", 1)        # K^T [D, S] bf16
        p_vtt = pool("p_vtt", 1)      # V^T staging before transpose
        p_vt = pool("p_vt", S // 128 + 4)  # V tiles [128 tok, D]
        p_at = pool("p_at", QH)       # attn^T per head [D, S] bf16
        p_exp = pool("p_exp", 16)     # exp(score) tiles bf16 (nk<=16 live)
        p_rt = pool("p_rt", 1)        # rope temps f32
        p_acc = pool("p_acc", 2)      # softmax-sum accumulators f32
        p_accb = pool("p_accb", 2)    # bf16 cast of acc (sums-matmul rhs)
        p_rb = pool("p_rb", 2)        # broadcast reciprocal
        p_ost = pool("p_ost", 4)      # output staging bf16
        p_pp = pool("p_pp", 5, "PSUM")    # general [128, QCH] psum ring
        p_av = pool("p_av", 2, "PSUM")    # attn-v accumulators
        p_sum = pool("p_sum", 1, "PSUM")  # softmax sum broadcast

        # ---- constants + weights (scalar queue; ht streams on sync queue) ----
        mask_sb = p_w.tile([128, 128], F32, name="mask_sb")
        eye_sb = p_w.tile([128, 128], BF16, name="eye_sb")
        ones2d = p_w.tile([128, 128], BF16, name="ones2d")
        wq_sb = p_w.tile([128, KT, FS], BF16, name="wq_sb")
        wk_sb = p_w.tile([128, KT, D], BF16, name="wk_sb")
        wv_sb = p_w.tile([128, KT, D], BF16, name="wv_sb")
        wo_sb = p_w.tile([128, QH, HID], BF16, name="wo_sb")
        cos_sb = p_w.tile([D, B, S], BF16, name="cos_sb")
        sin_sb = p_w.tile([D, B, S], BF16, name="sin_sb")
        nc.vector.memset(ones2d[:, :], 1.0)
        nc.scalar.dma_start(out=mask_sb[:, :], in_=mask[:, :])
        nc.scalar.dma_start(out=eye_sb[:, :], in_=eye[:, :])
        for kt in range(KT):
            nc.scalar.dma_start(out=wq_sb[:, kt, :], in_=wq[kt])
            nc.scalar.dma_start(out=wk_sb[:, kt, :], in_=wk[kt])
            nc.scalar.dma_start(out=wv_sb[:, kt, :], in_=wv[kt])
        for b in range(B):
            nc.scalar.dma_start(out=cos_sb[:, b, :], in_=trig[b, 0])
            nc.scalar.dma_start(out=sin_sb[:, b, :], in_=trig[b, 1])
        for f in range(QH):
            nc.scalar.dma_start(out=wo_sb[:, f, :], in_=wo[f])

        for b in range(B):
            def rope_evict(dst, ps, tci):
                # dst[:, sl] = ps * cos + rotate_half(ps) * sin  (write bf16)
                sl = slice(tci * QCH, (tci + 1) * QCH)
                cs = cos_sb[:, b, sl]
                sn = sin_sb[:, b, sl]
                t1 = p_rt.tile([128, QCH], F32, name="rt1", tag="rt1")
                t2 = p_rt.tile([128, QCH], F32, name="rt2", tag="rt2")
                nc.vector.tensor_mul(t1[:, :], ps[:, :], cs)
                nc.vector.tensor_mul(t2[0:HALF, :], ps[HALF:D, :], sn[0:HALF, :])
                nc.vector.tensor_mul(t2[HALF:D, :], ps[0:HALF, :], sn[HALF:D, :])
                nc.vector.tensor_sub(dst[0:HALF, sl], t1[0:HALF, :], t2[0:HALF, :])
                nc.vector.tensor_add(dst[HALF:D, sl], t1[HALF:D, :], t2[HALF:D, :])

            # ---- projections ----
            qtb = [p_qt.tile([D, S], BF16, name=f"qtb{f}", tag="qt") for f in range(QH)]
            ktb = p_kt.tile([D, S], BF16, name="ktb", tag="kt")
            vb = []
            for tci in range(TC):
                # KV sweep: 2 psums; then per-head Q sweeps (f-outer) so each
                # psum buf's eviction is covered by the next head's sweep.
                # p_pp sees 6 allocations per tci on a 5-ring (+1 drift), so
                # every allocation's predecessor eviction is ~a full sweep old.
                kps = p_pp.tile([128, QCH], F32, name="ps_k", tag="pp")
                vps = p_pp.tile([128, QCH], F32, name="ps_v", tag="pp")
                hts = []
                for kt in range(KT):
                    t = p_ht.tile([128, QCH], BF16, name="ht_t", tag="ht")
                    nc.sync.dma_start(out=t[:, :], in_=ht[b, kt, tci])
                    hts.append(t)
                    st_, sp_ = (kt == 0), (kt == KT - 1)
                    nc.tensor.matmul(kps[:, :], wk_sb[:, kt, :], t[:, :],
                                     start=st_, stop=sp_)
                    nc.tensor.matmul(vps[:, :], wv_sb[:, kt, :], t[:, :],
                                     start=st_, stop=sp_)
                vtt = p_vtt.tile([128, QCH], BF16, name="vtt", tag="vtt")
                nc.vector.tensor_copy(vtt[:, :], vps[:, :])
                qps = [p_pp.tile([128, QCH], F32, name=f"ps_q{f}", tag="pp")
                       for f in range(QH)]

                def q_sweep(f):
                    for kt in range(KT):
                        nc.tensor.matmul(qps[f][:, :],
                                         wq_sb[:, kt, f * D:(f + 1) * D],
                                         hts[kt][:, :],
                                         start=(kt == 0), stop=(kt == KT - 1))

                q_sweep(0)
                # V transpose on the p_av ring (PSUM) while q0 streams.
                for sub in range(QCH // 128):
                    pt = p_av.tile([128, 128], BF16, name="ps_vt", tag="pav")
                    nc.tensor.transpose(pt[:, :], vtt[:, sub * 128:(sub + 1) * 128],
                                        eye_sb[:, :])
                    v = p_vt.tile([128, D], BF16, name="v_t", tag="vt")
                    nc.vector.tensor_copy(v[:, :], pt[:, :])
                    vb.append(v)
                rope_evict(ktb, kps, tci)
                q_sweep(1)
                rope_evict(qtb[0], qps[0], tci)
                q_sweep(2)
                rope_evict(qtb[1], qps[1], tci)
                q_sweep(3)
                rope_evict(qtb[2], qps[2], tci)
                rope_evict(qtb[3], qps[3], tci)

            # ---- attention (scores kept transposed: [k, q]) ----
            # Per (h,qc) block: score matmuls + exps (PE ahead, ACT-paced),
            # then the AV chain; softmax-sum accumulation runs on GpSimd so
            # DVE only handles masks + the normalize tail. Each block's tail
            # (ones-matmul sum+broadcast -> fast reciprocal -> scale) is
            # emitted one block LATE so it never stalls the PE queue.
            atb = []
            pend = None  # (accb, av, at, qc) awaiting tail emission

            def emit_tail(pend):
                accb, av, at, qc = pend
                sums = p_sum.tile([128, QCH], F32, name="ps_sm", tag="sum")
                nc.tensor.matmul(sums[:, :], ones2d[:, :], accb[:, :],
                                 start=True, stop=True)
                rb = p_rb.tile([128, QCH], F32, name="rb", tag="rb")
                nc.vector.reciprocal_approx_fast(rb[:, :], sums[:, :])
                nc.vector.tensor_mul(at[:, qc * QCH:(qc + 1) * QCH],
                                     av[:, :], rb[:, :])

            for h in range(QH):
                at = p_at.tile([D, S], BF16, name=f"at{h}", tag="at")
                for qc in range(TC):
                    nk = KPQ * (qc + 1)
                    es = []
                    lows = []
                    for kt in range(nk):
                        j = kt - KPQ * qc           # >=0 inside diagonal band
                        lo = max(j, 0) * 128        # first live column
                        n = QCH - lo
                        lows.append(lo)
                        st = p_pp.tile([128, QCH], F32, name="ps_st", tag="pp")
                        nc.tensor.matmul(
                            st[:, 0:n], ktb[:, kt * 128:(kt + 1) * 128],
                            qtb[h][:, qc * QCH + lo:(qc + 1) * QCH],
                            start=True, stop=True)
                        if j >= 0:  # diagonal tile: triangular mask on 1st 128
                            nc.vector.tensor_add(st[:, 0:128], st[:, 0:128],
                                                 mask_sb[:, :])
                        e = p_exp.tile([128, QCH], BF16, name="e_t", tag="exp")
                        nc.scalar.activation(e[:, 0:n], st[:, 0:n], EXP, scale=SC)
                        es.append(e)
                    if pend is not None:
                        emit_tail(pend)
                    av = p_av.tile([128, QCH], F32, name="ps_av", tag="pav")
                    acc = p_acc.tile([128, QCH], F32, name="acc", tag="acc")
                    for kt in range(nk):
                        lo = lows[kt]
                        n = QCH - lo
                        if kt == 0:
                            nc.gpsimd.tensor_copy(acc[:, :], es[0][:, :])
                        else:
                            nc.gpsimd.tensor_add(acc[:, lo:QCH], acc[:, lo:QCH],
                                                 es[kt][:, 0:n])
                        nc.tensor.matmul(av[:, lo:QCH], vb[kt][:, :],
                                         es[kt][:, 0:n],
                                         start=(kt == 0), stop=(kt == nk - 1))
                    accb = p_accb.tile([128, QCH], BF16, name="accb", tag="accb")
                    nc.gpsimd.tensor_copy(accb[:, :], acc[:, :])
                    pend = (accb, av, at, qc)
                atb.append(at)
            emit_tail(pend)
            pend = None

            # ---- output projection (partial product; host sums across cores) ----
            for tt in range(TT):
                for hc in range(HC):
                    pw = p_pp.tile([128, QCH], F32, name="ps_wo", tag="pp")
                    for f in range(QH):
                        nc.tensor.matmul(
                            pw[:, :], atb[f][:, tt * 128:(tt + 1) * 128],
                            wo_sb[:, f, hc * QCH:(hc + 1) * QCH],
                            start=(f == 0), stop=(f == QH - 1))
                    o = p_ost.tile([128, QCH], BF16, name="o_t", tag="ost")
                    if (tt + hc) % 2 == 0:
                        nc.scalar.copy(o[:, :], pw[:, :])
                    else:
                        nc.vector.tensor_copy(o[:, :], pw[:, :])
                    nc.scalar.dma_start(out=out[b, tt, hc], in_=o[:, :])
    nc.finalize()  # Bacc: runs compile() (reg alloc, wait splitting) + freeze
    return nc


def shard_inputs(hidden_states, cos, sin, Wq, Wk, Wv, Wo, n_cores=N_CORES,
                 QH=4, D=128, QCH=512):
    """Host-side prep: transpose/tile/bf16-round everything per core."""
    bf16 = ml_dtypes.bfloat16
    B, S, HID = hidden_states.shape
    FS = QH * D
    KT = HID // 128
    TC = S // QCH

    hT = hidden_states.astype(bf16).transpose(0, 2, 1)           # [B, HID, S]
    ht_t = np.ascontiguousarray(
        hT.reshape(B, KT, 128, TC, QCH).transpose(0, 1, 3, 2, 4))
    trig = np.ascontiguousarray(np.stack(
        [cos.transpose(0, 2, 1), sin.transpose(0, 2, 1)], axis=1)
    ).astype(bf16)

    kk = np.arange(128)[:, None]
    cc = np.arange(128)[None, :]
    maskv = np.where(cc < kk, np.float32(NEG), np.float32(0.0))
    eyev = np.eye(128, dtype=bf16)

    in_maps = []
    for i in range(n_cores):
        wq_i = Wq[i * FS:(i + 1) * FS, :].T.astype(bf16).reshape(KT, 128, FS)
        wk_i = Wk[i * D:(i + 1) * D, :].T.astype(bf16).reshape(KT, 128, D)
        wv_i = Wv[i * D:(i + 1) * D, :].T.astype(bf16).reshape(KT, 128, D)
        wo_i = Wo[:, i * FS:(i + 1) * FS].T.astype(bf16).reshape(QH, D, HID)
        in_maps.append(dict(ht=ht_t, trig=trig, wq=wq_i, wk=wk_i, wv=wv_i,
                            wo=wo_i, mask=maskv, eye=eyev))
    return in_maps


_NC_CACHE = {}


def kernel(hidden_states, cos, sin, Wq, Wk, Wv, Wo, _trace=False):
    hidden_states = np.asarray(hidden_states)
    cos = np.asarray(cos)
    sin = np.asarray(sin)
    Wq, Wk, Wv, Wo = (np.asarray(a) for a in (Wq, Wk, Wv, Wo))
    B, S, HID = hidden_states.shape
    QCH = 512

    key = (B, S, HID)
    nc = _NC_CACHE.get(key)
    if nc is None:
        nc = _NC_CACHE[key] = build_core_kernel(B, S, HID)

    in_maps = shard_inputs(hidden_states, cos, sin, Wq, Wk, Wv, Wo)
    res = run_bass_kernel_spmd(nc, in_maps, core_ids=list(range(N_CORES)),
                               trace=_trace)
    kernel._last_results = res

    acc = res.results[0]["out"].astype(np.float32)
    for r in res.results[1:]:
        acc = acc + r["out"].astype(np.float32)
    # [B, TT, HC, 128, QCH] -> [B, S, HID]
    TT = S // 128
    HC = HID // QCH
    full = acc.transpose(0, 1, 3, 2, 4).reshape(B, S, HID)
    return np.ascontiguousarray(full)


# revision 8
# speedup vs baseline: 1.2785x; 1.2785x over previous
# Trainium2 Bass kernel: Llama-style attention block (GQA + RoPE + causal),
# tensor-parallel across heads on 8 NeuronCores.
#
# Full-shape contract: kernel(**inputs) takes the unsharded numpy inputs and
# returns the full [B, S, HID] float32 output.
#
# Sharding strategy (per core i of 8):
#   - 4 query heads (rows i*512:(i+1)*512 of Wq) + 1 kv head (rows i*128.. of Wk/Wv)
#   - Wo is sharded row-wise (its columns i*512:(i+1)*512); each core emits a
#     partial [B,S,HID] product (bf16) which the host sums at gather time.
# All weights/activations are pre-transposed and pre-tiled on the host so the
# device kernel needs zero on-chip transposes of activations:
#   matmul(out[M,N], lhsT[K,M], rhs[K,N]) contracts over the partition dim K.
# Compute in bf16 (fp32 PSUM accumulation); scores stay transposed [k, q];
# softmax is unnormalized (scores are O(10), exp safe in fp32); the softmax
# row-sum is accumulated on DVE during the kt loop, then sum+broadcast happen
# in ONE [128,128]-ones matmul, reciprocal via the fast DVE approx, folded in
# at the attention-output eviction. The causal diagonal band computes only
# live columns.
#
# Phase structure per (b, tci):  KV-sweep (2 psums) -> Q-sweep (4 psums),
# so the rope/copy evictions of each sweep hide behind the other sweep's
# matmuls instead of stalling the next chunk's accumulation.

import os
import sys
from contextlib import ExitStack

for _p in ("/opt/trn_rl_repo", "/root/.axon_site/_ro/trn_rl_repo"):
    if os.path.isdir(_p) and _p not in sys.path:
        sys.path.append(_p)

import ml_dtypes
import numpy as np

import concourse.bass as bass
import concourse.mybir as mybir
import concourse.tile as tile
from concourse import bacc
from concourse.bass_utils import run_bass_kernel_spmd

BF16 = mybir.dt.bfloat16
F32 = mybir.dt.float32
NEG = -1.0e9
N_CORES = 8


def build_core_kernel(B, S, HID, QH=4, D=128, QCH=512):
    """SPMD per-core program. QH query heads + 1 kv head per core.

    DRAM parameter layouts (host pre-tiles everything):
      ht   [B, KT, TC, 128, QCH]  bf16  hidden^T tiles: [b,kt,tc,i,j] = hidden[b, tc*QCH+j, kt*128+i]
      trig [B, 2, D, S]           bf16  cos^T / sin^T
      wq   [KT, 128, QH*D]        bf16  Wq_core^T tiles (contraction rows blocked by 128)
      wk   [KT, 128, D]           bf16
      wv   [KT, 128, D]           bf16
      wo   [QH, D, HID]           bf16  Wo_core^T rows blocked per head
      mask [128, 128]             f32   additive causal triangle
      eye  [128, 128]             bf16
      out  [B, TT, HC, 128, QCH]  bf16  partial output tiles
    """
    FS = QH * D          # per-core feature slice of the qkv/attn space
    KT = HID // 128      # contraction tiles for projections
    TC = S // QCH        # 512-token chunks
    TT = S // 128        # 128-token tiles
    HC = HID // QCH      # output hid chunks
    KPQ = QCH // 128     # k-tiles per q-chunk (diagonal band width)
    HALF = D // 2
    SC = float(1.0 / np.sqrt(D))
    EXP = mybir.ActivationFunctionType.Exp

    # Bacc (not plain Bass): its compile pipeline splits multi-sem waits into
    # EventSemaphore instructions — the DMA DIRECT2D struct has one wait slot.
    nc = bacc.Bacc(None)
    ht = nc.declare_dram_parameter("ht", [B, KT, TC, 128, QCH], BF16, isOutput=False)
    trig = nc.declare_dram_parameter("trig", [B, 2, D, S], BF16, isOutput=False)
    wq = nc.declare_dram_parameter("wq", [KT, 128, FS], BF16, isOutput=False)
    wk = nc.declare_dram_parameter("wk", [KT, 128, D], BF16, isOutput=False)
    wv = nc.declare_dram_parameter("wv", [KT, 128, D], BF16, isOutput=False)
    wo = nc.declare_dram_parameter("wo", [QH, D, HID], BF16, isOutput=False)
    mask = nc.declare_dram_parameter("mask", [128, 128], F32, isOutput=False)
    eye = nc.declare_dram_parameter("eye", [128, 128], BF16, isOutput=False)
    out = nc.declare_dram_parameter("out", [B, TT, HC, 128, QCH], BF16, isOutput=True)

    with ExitStack() as ctx:
        tc = ctx.enter_context(tile.TileContext(nc))
        pool = lambda name, bufs, space=None: ctx.enter_context(
            tc.tile_pool(name=name, bufs=bufs, **({"space": space} if space else {}))
        )
        p_w = pool("p_w", 1)          # weights + constants, loaded once
        p_ht = pool("p_ht", 33)       # hidden^T tiles, resident across both sweeps
        p_qt = pool("p_qt", QH)       # per-head Q^T [D, S] bf16
        p_kt = pool("p_kt", 1)        # K^T [D, S] bf16
        p_vtt = pool("p_vtt", 1)      # V^T staging before transpose
        p_vt = pool("p_vt", S // 128 + 4)  # V tiles [128 tok, D]
        p_at = pool("p_at", QH)       # attn^T per head [D, S] bf16
        p_exp = pool("p_exp", 17)     # exp(score) tiles bf16 (nk<=16 live)
        p_rt = pool("p_rt", 1)        # rope temps f32
        p_acc = pool("p_acc", 2)      # softmax-sum accumulators bf16
        p_rb = pool("p_rb", 2)        # broadcast reciprocal
        p_ost = pool("p_ost", 4)      # output staging bf16
        p_pp = pool("p_pp", 5, "PSUM")    # general [128, QCH] psum ring
        p_av = pool("p_av", 2, "PSUM")    # attn-v accumulators
        p_sum = pool("p_sum", 1, "PSUM")  # softmax sum broadcast

        # ---- constants + weights (scalar queue; ht streams on sync queue) ----
        mask_sb = p_w.tile([128, 128], F32, name="mask_sb")
        eye_sb = p_w.tile([128, 128], BF16, name="eye_sb")
        ones2d = p_w.tile([128, 128], BF16, name="ones2d")
        wq_sb = p_w.tile([128, KT, FS], BF16, name="wq_sb")
        wk_sb = p_w.tile([128, KT, D], BF16, name="wk_sb")
        wv_sb = p_w.tile([128, KT, D], BF16, name="wv_sb")
        wo_sb = p_w.tile([128, QH, HID], BF16, name="wo_sb")
        cos_sb = p_w.tile([D, B, S], BF16, name="cos_sb")
        sin_sb = p_w.tile([D, B, S], BF16, name="sin_sb")
        nc.vector.memset(ones2d[:, :], 1.0)
        nc.scalar.dma_start(out=mask_sb[:, :], in_=mask[:, :])
        nc.scalar.dma_start(out=eye_sb[:, :], in_=eye[:, :])
        for kt in range(KT):
            nc.scalar.dma_start(out=wq_sb[:, kt, :], in_=wq[kt])
            nc.scalar.dma_start(out=wk_sb[:, kt, :], in_=wk[kt])
            nc.scalar.dma_start(out=wv_sb[:, kt, :], in_=wv[kt])
        for b in range(B):
            nc.scalar.dma_start(out=cos_sb[:, b, :], in_=trig[b, 0])
            nc.scalar.dma_start(out=sin_sb[:, b, :], in_=trig[b, 1])
        for f in range(QH):
            nc.scalar.dma_start(out=wo_sb[:, f, :], in_=wo[f])

        for b in range(B):
            def rope_evict(dst, ps, tci):
                # dst[:, sl] = ps * cos + rotate_half(ps) * sin  (write bf16)
                sl = slice(tci * QCH, (tci + 1) * QCH)
                cs = cos_sb[:, b, sl]
                sn = sin_sb[:, b, sl]
                t1 = p_rt.tile([128, QCH], F32, name="rt1", tag="rt1")
                t2 = p_rt.tile([128, QCH], F32, name="rt2", tag="rt2")
                nc.vector.tensor_mul(t1[:, :], ps[:, :], cs)
                nc.vector.tensor_mul(t2[0:HALF, :], ps[HALF:D, :], sn[0:HALF, :])
                nc.vector.tensor_mul(t2[HALF:D, :], ps[0:HALF, :], sn[HALF:D, :])
                nc.vector.tensor_sub(dst[0:HALF, sl], t1[0:HALF, :], t2[0:HALF, :])
                nc.vector.tensor_add(dst[HALF:D, sl], t1[HALF:D, :], t2[HALF:D, :])

            # ---- projections ----
            qtb = [p_qt.tile([D, S], BF16, name=f"qtb{f}", tag="qt") for f in range(QH)]
            ktb = p_kt.tile([D, S], BF16, name="ktb", tag="kt")
            vb = []
            for tci in range(TC):
                # KV sweep: 2 psums; then per-head Q sweeps (f-outer) so each
                # psum buf's eviction is covered by the next head's sweep.
                # p_pp sees 6 allocations per tci on a 5-ring (+1 drift), so
                # every allocation's predecessor eviction is ~a full sweep old.
                kps = p_pp.tile([128, QCH], F32, name="ps_k", tag="pp")
                vps = p_pp.tile([128, QCH], F32, name="ps_v", tag="pp")
                hts = []
                for kt in range(KT):
                    t = p_ht.tile([128, QCH], BF16, name="ht_t", tag="ht")
                    nc.sync.dma_start(out=t[:, :], in_=ht[b, kt, tci])
                    hts.append(t)
                    st_, sp_ = (kt == 0), (kt == KT - 1)
                    nc.tensor.matmul(kps[:, :], wk_sb[:, kt, :], t[:, :],
                                     start=st_, stop=sp_)
                    nc.tensor.matmul(vps[:, :], wv_sb[:, kt, :], t[:, :],
                                     start=st_, stop=sp_)
                vtt = p_vtt.tile([128, QCH], BF16, name="vtt", tag="vtt")
                nc.vector.tensor_copy(vtt[:, :], vps[:, :])
                qps = [p_pp.tile([128, QCH], F32, name=f"ps_q{f}", tag="pp")
                       for f in range(QH)]

                def q_sweep(f):
                    for kt in range(KT):
                        nc.tensor.matmul(qps[f][:, :],
                                         wq_sb[:, kt, f * D:(f + 1) * D],
                                         hts[kt][:, :],
                                         start=(kt == 0), stop=(kt == KT - 1))

                q_sweep(0)
                # V transpose on the p_av ring (PSUM) while q0 streams.
                for sub in range(QCH // 128):
                    pt = p_av.tile([128, 128], BF16, name="ps_vt", tag="pav")
                    nc.tensor.transpose(pt[:, :], vtt[:, sub * 128:(sub + 1) * 128],
                                        eye_sb[:, :])
                    v = p_vt.tile([128, D], BF16, name="v_t", tag="vt")
                    nc.vector.tensor_copy(v[:, :], pt[:, :])
                    vb.append(v)
                rope_evict(ktb, kps, tci)
                q_sweep(1)
                rope_evict(qtb[0], qps[0], tci)
                q_sweep(2)
                rope_evict(qtb[1], qps[1], tci)
                q_sweep(3)
                rope_evict(qtb[2], qps[2], tci)
                rope_evict(qtb[3], qps[3], tci)

            # ---- attention (scores kept transposed: [k, q]) ----
            # Per (h,qc) block: score matmuls + exps (PE ahead, ACT-paced),
            # then the AV chain; the softmax-sum accumulates on DVE in bf16
            # (2x mode, and the sums-matmul reads it directly). Each block's
            # tail (ones-matmul sum+broadcast -> fast reciprocal -> scale) is
            # emitted one block LATE so it never stalls the PE queue.
            atb = []
            pend = None  # (acc, av, at, qc) awaiting tail emission

            def emit_tail(pend):
                acc, av, at, qc = pend
                sums = p_sum.tile([128, QCH], F32, name="ps_sm", tag="sum")
                nc.tensor.matmul(sums[:, :], ones2d[:, :], acc[:, :],
                                 start=True, stop=True)
                rb = p_rb.tile([128, QCH], F32, name="rb", tag="rb")
                nc.vector.reciprocal_approx_fast(rb[:, :], sums[:, :])
                nc.vector.tensor_mul(at[:, qc * QCH:(qc + 1) * QCH],
                                     av[:, :], rb[:, :])

            for h in range(QH):
                at = p_at.tile([D, S], BF16, name=f"at{h}", tag="at")
                for qc in range(TC):
                    nk = KPQ * (qc + 1)
                    es = []
                    lows = []
                    for kt in range(nk):
                        j = kt - KPQ * qc           # >=0 inside diagonal band
                        lo = max(j, 0) * 128        # first live column
                        n = QCH - lo
                        lows.append(lo)
                        st = p_pp.tile([128, QCH], F32, name="ps_st", tag="pp")
                        nc.tensor.matmul(
                            st[:, 0:n], ktb[:, kt * 128:(kt + 1) * 128],
                            qtb[h][:, qc * QCH + lo:(qc + 1) * QCH],
                            start=True, stop=True)
                        if j >= 0:  # diagonal tile: triangular mask on 1st 128
                            nc.vector.tensor_add(st[:, 0:128], st[:, 0:128],
                                                 mask_sb[:, :])
                        e = p_exp.tile([128, QCH], BF16, name="e_t", tag="exp")
                        nc.scalar.activation(e[:, 0:n], st[:, 0:n], EXP, scale=SC)
                        es.append(e)
                    if pend is not None:
                        emit_tail(pend)
                    av = p_av.tile([128, QCH], F32, name="ps_av", tag="pav")
                    acc = p_acc.tile([128, QCH], BF16, name="acc", tag="acc")
                    for kt in range(nk):
                        lo = lows[kt]
                        n = QCH - lo
                        if kt == 0:
                            nc.vector.tensor_copy(acc[:, :], es[0][:, :])
                        else:
                            nc.vector.tensor_add(acc[:, lo:QCH], acc[:, lo:QCH],
                                                 es[kt][:, 0:n])
                        nc.tensor.matmul(av[:, lo:QCH], vb[kt][:, :],
                                         es[kt][:, 0:n],
                                         start=(kt == 0), stop=(kt == nk - 1))
                    pend = (acc, av, at, qc)
                atb.append(at)
            emit_tail(pend)
            pend = None

            # ---- output projection (partial product; host sums across cores) ----
            for tt in range(TT):
                for hc in range(HC):
                    pw = p_pp.tile([128, QCH], F32, name="ps_wo", tag="pp")
                    for f in range(QH):
                        nc.tensor.matmul(
                            pw[:, :], atb[f][:, tt * 128:(tt + 1) * 128],
                            wo_sb[:, f, hc * QCH:(hc + 1) * QCH],
                            start=(f == 0), stop=(f == QH - 1))
                    o = p_ost.tile([128, QCH], BF16, name="o_t", tag="ost")
                    if (tt + hc) % 2 == 0:
                        nc.scalar.copy(o[:, :], pw[:, :])
                    else:
                        nc.vector.tensor_copy(o[:, :], pw[:, :])
                    nc.scalar.dma_start(out=out[b, tt, hc], in_=o[:, :])
    nc.finalize()  # Bacc: runs compile() (reg alloc, wait splitting) + freeze
    return nc


def shard_inputs(hidden_states, cos, sin, Wq, Wk, Wv, Wo, n_cores=N_CORES,
                 QH=4, D=128, QCH=512):
    """Host-side prep: transpose/tile/bf16-round everything per core."""
    bf16 = ml_dtypes.bfloat16
    B, S, HID = hidden_states.shape
    FS = QH * D
    KT = HID // 128
    TC = S // QCH

    hT = hidden_states.astype(bf16).transpose(0, 2, 1)           # [B, HID, S]
    ht_t = np.ascontiguousarray(
        hT.reshape(B, KT, 128, TC, QCH).transpose(0, 1, 3, 2, 4))
    trig = np.ascontiguousarray(np.stack(
        [cos.transpose(0, 2, 1), sin.transpose(0, 2, 1)], axis=1)
    ).astype(bf16)

    kk = np.arange(128)[:, None]
    cc = np.arange(128)[None, :]
    maskv = np.where(cc < kk, np.float32(NEG), np.float32(0.0))
    eyev = np.eye(128, dtype=bf16)

    in_maps = []
    for i in range(n_cores):
        wq_i = Wq[i * FS:(i + 1) * FS, :].T.astype(bf16).reshape(KT, 128, FS)
        wk_i = Wk[i * D:(i + 1) * D, :].T.astype(bf16).reshape(KT, 128, D)
        wv_i = Wv[i * D:(i + 1) * D, :].T.astype(bf16).reshape(KT, 128, D)
        wo_i = Wo[:, i * FS:(i + 1) * FS].T.astype(bf16).reshape(QH, D, HID)
        in_maps.append(dict(ht=ht_t, trig=trig, wq=wq_i, wk=wk_i, wv=wv_i,
                            wo=wo_i, mask=maskv, eye=eyev))
    return in_maps


_NC_CACHE = {}


def kernel(hidden_states, cos, sin, Wq, Wk, Wv, Wo, _trace=False):
    hidden_states = np.asarray(hidden_states)
    cos = np.asarray(cos)
    sin = np.asarray(sin)
    Wq, Wk, Wv, Wo = (np.asarray(a) for a in (Wq, Wk, Wv, Wo))
    B, S, HID = hidden_states.shape
    QCH = 512

    key = (B, S, HID)
    nc = _NC_CACHE.get(key)
    if nc is None:
        nc = _NC_CACHE[key] = build_core_kernel(B, S, HID)

    in_maps = shard_inputs(hidden_states, cos, sin, Wq, Wk, Wv, Wo)
    res = run_bass_kernel_spmd(nc, in_maps, core_ids=list(range(N_CORES)),
                               trace=_trace)
    kernel._last_results = res

    acc = res.results[0]["out"].astype(np.float32)
    for r in res.results[1:]:
        acc = acc + r["out"].astype(np.float32)
    # [B, TT, HC, 128, QCH] -> [B, S, HID]
    TT = S // 128
    HC = HID // QCH
    full = acc.transpose(0, 1, 3, 2, 4).reshape(B, S, HID)
    return np.ascontiguousarray(full)
